# revision 1
# baseline (speedup 1.0000x reference)
"""Trainium2 Bass kernel for nn_Convolution (e3nn-style GNN message passing).

Strategy (8 NeuronCores, SPMD, no collectives):
- Sort edges by destination; core c owns destination nodes [6400c, 6400(c+1)).
- Per core: edges are binned into 50 node-blocks (128 nodes each) and padded to
  NG groups of 128 edges per block. Dummy edges gather a zero table row, so
  every tensor-product output term (all bilinear in source features) is 0.
- Gather source features with dma_gather from a 256B-padded table, split into
  lo/hi halves (int16 index limit), summed on DVE.
- Radial MLP layer 1 on PE with tile_position row-packed K=8 matmuls,
  layer 2 per-group with h as the stationary operand (w lands [edge, 256]).
- TP products on DVE via broadcast APs; the per-edge contraction over u is
  DEFERRED into the scatter matmul: one-hot(dst) x [512-wide product tile]
  accumulates in PSUM over each block, reduced over u once per block.
"""

import math
import os
import numpy as np

_TRACE_SIM = bool(int(os.environ.get('K_TRACE_SIM', '0')))
_NO_GATHER = bool(int(os.environ.get('K_NO_GATHER', '0')))
_NO_TP = bool(int(os.environ.get('K_NO_TP', '0')))
_NO_MM = bool(int(os.environ.get('K_NO_MM', '0')))


import concourse.bass as bass
import concourse.bacc as bacc
import concourse.mybir as mybir
from concourse.tile import TileContext
from concourse.bass_utils import run_bass_kernel_spmd

# ---------------- problem constants (hardcoded per spec) ----------------
N_NODES, N_EDGES, NUM_BASIS, HIDDEN = 50000, 800000, 8, 256
MUL = 8
INV_SQRT3 = float(1.0 / np.sqrt(3.0))
A_SCALAR = float(np.sqrt(1.0 / 128.0))
A_VECTOR = float(np.sqrt(3.0 / 128.0))
SQRT2 = float(np.sqrt(2.0))
DEG_SCALE = float(1.0 / np.sqrt(N_EDGES / N_NODES))

NCORES = 8
P = 128
NODES_PER_CORE = 6400          # 50 blocks of 128; 8*6400 = 51200 >= 50000
NB = 50                        # node blocks per core
# table: rows 1..50000 = nodes 0..49999; row 50001 = zeros (dummy target).
# gather base = row 32768, int16 idx = node - 32767 in [-32767, 17232];
# dummy idx = +17233 (always non-negative so it never hits the trailing-
# negative trim). Each gather's last (trim-order) index is forced >= 0 by an
# in-block edge swap on the host.
TBL_ROWS = 50004
GBASE = 32768
DUMMY_IDX = 50001 - GBASE

_PROG_CACHE = {}


# ---------------- device program ----------------
def _build_program(NG):
    GROUPS = NB * NG
    WINDOWS = GROUPS // 4            # 4 groups (512 edges) per window
    SUPER_G = 4                      # one gather per window (512 idx <= 1024 cap)
    NSUPER = GROUPS // SUPER_G
    IDXW = SUPER_G * 128 // 16       # wrapped idx cols per super
    NJ = (WINDOWS + 3) // 4          # es_w4 column blocks
    ES_CHUNK_J = 3                   # es col-blocks per streamed chunk
    NCHUNK = (NJ + ES_CHUNK_J - 1) // ES_CHUNK_J

    nc = bacc.Bacc(num_devices=NCORES, num_swdge_queues=4)
    f32, i16 = mybir.dt.float32, mybir.dt.int16

    tbl = nc.dram_tensor("tbl", [TBL_ROWS, 64], f32, kind="ExternalInput")
    idx_g = nc.dram_tensor("idx_g", [P, NSUPER * IDXW], i16, kind="ExternalInput")
    es4 = nc.dram_tensor("es4", [P, NJ * 512], f32, kind="ExternalInput")
    sh_t = nc.dram_tensor("sh_t", [P, GROUPS * 4], f32, kind="ExternalInput")
    dstl = nc.dram_tensor("dstl", [P, GROUPS], f32, kind="ExternalInput")
    w1t = nc.dram_tensor("w1t", [P, 256], f32, kind="ExternalInput")
    w2t = nc.dram_tensor("w2t", [P, 512], f32, kind="ExternalInput")
    iota = nc.dram_tensor("iota", [P, P], f32, kind="ExternalInput")
    nodeout = nc.dram_tensor("nodeout", [NODES_PER_CORE, 32], f32, kind="ExternalOutput")

    AX = mybir.AxisListType.X
    ADD = mybir.AluOpType.add
    MUL_ = mybir.AluOpType.mult
    EQ = mybir.AluOpType.is_equal
    RELU = mybir.ActivationFunctionType.Relu

    with TileContext(nc, trace_sim=_TRACE_SIM) as tc:
        with tc.tile_pool(name="const", bufs=1) as cpool, \
             tc.tile_pool(name="stream", bufs=2) as spool, \
             tc.tile_pool(name="work", bufs=2) as wpool, \
             tc.tile_pool(name="oh", bufs=3) as ohpool, \
             tc.tile_pool(name="psum", bufs=2, space="PSUM") as pp, \
             tc.tile_pool(name="psum1", bufs=1, space="PSUM") as pp1:

            # constants resident in SBUF
            ig_sb = cpool.tile([P, NSUPER * IDXW], i16, name="ig")
            nc.sync.dma_start(ig_sb[:], idx_g[:])
            sh_sb = cpool.tile([P, GROUPS, 4], f32, name="sh")
            nc.sync.dma_start(sh_sb[:], sh_t[:].rearrange("p (g k) -> p g k", k=4))
            dl_sb = cpool.tile([P, GROUPS], f32, name="dl")
            nc.sync.dma_start(dl_sb[:], dstl[:])
            w1_sb = cpool.tile([P, 256], f32, name="w1")
            nc.sync.dma_start(w1_sb[:], w1t[:])
            w2_sb = cpool.tile([P, 2, 256], f32, name="w2")
            nc.sync.dma_start(w2_sb[:], w2t[:].rearrange("p (h n) -> p h n", h=2))
            io_sb = cpool.tile([P, P], f32, name="iota")
            nc.sync.dma_start(io_sb[:], iota[:])

            acc_ps = None
            for w in range(WINDOWS):
                c = w % 4
                j = w // 4
                g0 = 4 * w

                # --- stream es chunk (every ES_CHUNK_J col-blocks)
                if j % ES_CHUNK_J == 0:
                    jw = min(ES_CHUNK_J, NJ - j)
                    es_sb = spool.tile([P, ES_CHUNK_J * 512], f32, tag="es")
                    nc.sync.dma_start(
                        es_sb[:, : jw * 512], es4[:, j * 512 : (j + jw) * 512]
                    )
                jj = j % ES_CHUNK_J

                # --- gather: one 512-idx call per window, cycling queues
                x_c = spool.tile([P, 4, 64], f32, tag="xc")
                if _NO_GATHER:
                    nc.vector.memset(x_c[:, :, 0:4], 0.0)
                elif True:
                    nc.gpsimd.dma_gather(
                    out_ap=x_c[:], in_ap=tbl[GBASE:, :],
                    idxs_ap=ig_sb[:, w * IDXW : (w + 1) * IDXW],
                    num_idxs=512, num_idxs_reg=512, elem_size=64,
                    queue_num=w % 4,
                )
                # --- MLP1: h[comp, edge] for 512 edges, two 128-comp halves
                h_ps = pp1.tile([P, 2, 512], f32, space="PSUM", tag="hps")
                for half in range(2):
                    nc.tensor.matmul(
                        h_ps[:, half, :],
                        lhsT=w1_sb[32 * c : 32 * c + 8, half * 128 : half * 128 + 128],
                        rhs=es_sb[32 * c : 32 * c + 8, jj * 512 : jj * 512 + 512],
                        start=True, stop=True,
                        tile_position=(32 * c, 0),
                    )
                h_sb = wpool.tile([P, 2, 512], f32, tag="hsb")
                for half in range(2):
                    nc.scalar.activation(
                        out=h_sb[:, half, :], in_=h_ps[:, half, :], func=RELU
                    )

                # --- MLP2 per group: w[edge, 256] in PSUM
                w_ps = pp.tile([P, 4, 256], f32, space="PSUM", tag="wps")
                for gg in range(4):
                    for half in range(2):
                        nc.tensor.matmul(
                            w_ps[:, gg, :],
                            lhsT=h_sb[:, half, gg * 128 : gg * 128 + 128],
                            rhs=w2_sb[:, half, :],
                            start=(half == 0), stop=(half == 1),
                        )

                # --- TP products (batched over the 4 groups)
                xs = x_c[:]                           # [P, 4, 64]
                shw = sh_sb[:, g0 : g0 + 4, :]        # [P, 4, 4]
                scat = wpool.tile([P, 4, 512], f32, tag="scat")
                ab16 = wpool.tile([P, 4, 16], f32, tag="ab16")
                # a[u] = s1[u] * s2
                nc.vector.tensor_tensor(
                    out=ab16[:, :, 0:8], in0=xs[:, :, 0:8],
                    in1=shw[:, :, 0:1].to_broadcast([P, 4, 8]), op=MUL_,
                )
                # b[u] = sum_i v1[u,i] * v2[i]
                pb = wpool.tile([P, 4, 8, 3], f32, tag="pb")
                nc.vector.tensor_tensor(
                    out=pb[:],
                    in0=xs[:, :, 8:32].rearrange("p g (u i) -> p g u i", u=8),
                    in1=shw[:, :, 1:4].unsqueeze(2).to_broadcast([P, 4, 8, 3]),
                    op=MUL_,
                )
                nc.vector.tensor_reduce(
                    out=ab16[:, :, 8:16], in_=pb[:], axis=AX, op=ADD
                )
                # ps = ab16[u'] * w01[u', w_]  -> scat[:, :, 0:128]
                nc.vector.tensor_tensor(
                    out=scat[:, :, 0:128].rearrange("p g (u w) -> p g u w", u=16),
                    in0=ab16[:].unsqueeze(3).to_broadcast([P, 4, 16, 8]),
                    in1=w_ps[:, :, 0:128].rearrange("p g (u w) -> p g u w", u=16),
                    op=MUL_,
                )
                # s1v2[u,i] = s1[u]*v2[i]
                s1v2 = wpool.tile([P, 4, 24], f32, tag="s1v2")
                nc.vector.tensor_tensor(
                    out=s1v2[:].rearrange("p g (u i) -> p g u i", u=8),
                    in0=xs[:, :, 0:8].unsqueeze(3).to_broadcast([P, 4, 8, 3]),
                    in1=shw[:, :, 1:4].unsqueeze(2).to_broadcast([P, 4, 8, 3]),
                    op=MUL_,
                )
                # ptv = s1v2[u,i] * w2[u,w_] -> scat cols 128:320 ((u,w_,i))
                nc.vector.tensor_tensor(
                    out=scat[:, :, 128:320].rearrange(
                        "p g (u w i) -> p g u w i", u=8, w=8
                    ),
                    in0=s1v2[:].rearrange("p g (u i) -> p g u i", u=8)
                        .unsqueeze(3).to_broadcast([P, 4, 8, 8, 3]),
                    in1=w_ps[:, :, 128:192]
                        .rearrange("p g (u w) -> p g u w", u=8)
                        .unsqueeze(4).to_broadcast([P, 4, 8, 8, 3]),
                    op=MUL_,
                )
                # v1s2[u,i] = v1[u,i]*s2
                v1s2 = wpool.tile([P, 4, 24], f32, tag="v1s2")
                nc.vector.tensor_tensor(
                    out=v1s2[:], in0=xs[:, :, 8:32],
                    in1=shw[:, :, 0:1].to_broadcast([P, 4, 24]), op=MUL_,
                )
                # pv3 = v1s2[u,i] * w3[u,w_] -> scat cols 320:512
                nc.vector.tensor_tensor(
                    out=scat[:, :, 320:512].rearrange(
                        "p g (u w i) -> p g u w i", u=8, w=8
                    ),
                    in0=v1s2[:].rearrange("p g (u i) -> p g u i", u=8)
                        .unsqueeze(3).to_broadcast([P, 4, 8, 8, 3]),
                    in1=w_ps[:, :, 192:256]
                        .rearrange("p g (u w) -> p g u w", u=8)
                        .unsqueeze(4).to_broadcast([P, 4, 8, 8, 3]),
                    op=MUL_,
                )

                # --- per group: one-hot + scatter matmul into block accumulator
                for gg in range(4):
                    g = g0 + gg
                    b = g // NG
                    gib = g % NG
                    if gib == 0:
                        acc_ps = pp.tile([P, 512], f32, space="PSUM", tag="acc")
                    oh = ohpool.tile([P, P], f32, tag="oh")
                    nc.vector.tensor_scalar(
                        out=oh[:], in0=io_sb[:], scalar1=dl_sb[:, g : g + 1],
                        scalar2=None, op0=EQ,
                    )
                    nc.tensor.matmul(
                        acc_ps[:],
                        lhsT=oh[:], rhs=scat[:, gg, :],
                        start=(gib == 0), stop=(gib == NG - 1),
                    )
                    if gib == NG - 1:
                        stage = wpool.tile([P, 32], f32, tag="stage")
                        nc.vector.tensor_reduce(
                            out=stage[:, 0:8],
                            in_=acc_ps[:, 0:128].rearrange(
                                "p (u w) -> p w u", u=16
                            ),
                            axis=AX, op=ADD,
                        )
                        nc.vector.tensor_reduce(
                            out=stage[:, 8:32],
                            in_=acc_ps[:, 128:512].rearrange(
                                "p (u wi) -> p wi u", u=16
                            ),
                            axis=AX, op=ADD,
                        )
                        nc.sync.dma_start(
                            nodeout[128 * b : 128 * b + 128, :], stage[:]
                        )
    nc.compile()
    return nc


# ---------------- host-side prep ----------------
def _prep(node_features, edge_src, edge_dst, edge_sh, edge_scalars, fc_w1, fc_w2, NG):
    GROUPS = NB * NG
    EPAD = GROUPS * 128
    WINDOWS = GROUPS // 4
    SUPER_G = 4
    NSUPER = GROUPS // SUPER_G
    IDXW = SUPER_G * 128 // 16
    NJ = (WINDOWS + 3) // 4

    # fold all scalar coefficients into the weights
    w1s = (fc_w1 * (1.0 / math.sqrt(NUM_BASIS))).astype(np.float32)     # [8, 256]
    w2 = (fc_w2 * (SQRT2 / math.sqrt(HIDDEN))).astype(np.float64)       # [256, 256]
    w2 = w2.reshape(HIDDEN, 4, MUL, MUL)
    coef = np.array(
        [A_SCALAR, A_SCALAR * INV_SQRT3, A_VECTOR * INV_SQRT3, A_VECTOR * INV_SQRT3]
    ) * DEG_SCALE
    w2 = w2 * coef[None, :, None, None]
    # device col order: [w01 (u'16, w8) | w2 (u8, w8) | w3 (u8, w8)]
    w2dev = np.concatenate(
        [
            w2[:, 0].reshape(HIDDEN, 64),
            w2[:, 1].reshape(HIDDEN, 64),
            w2[:, 2].reshape(HIDDEN, 64),
            w2[:, 3].reshape(HIDDEN, 64),
        ],
        axis=1,
    ).astype(np.float32)                                                # [256, 256]

    w1t = np.zeros((P, 256), np.float32)
    for c in range(4):
        w1t[32 * c : 32 * c + 8] = w1s
    w2t = np.zeros((P, 512), np.float32)
    w2t[:, 0:256] = w2dev[0:128]
    w2t[:, 256:512] = w2dev[128:256]
    iota = np.broadcast_to(np.arange(P, dtype=np.float32), (P, P)).copy()

    tbl = np.zeros((TBL_ROWS, 64), np.float32)
    tbl[1 : N_NODES + 1, 0:32] = node_features

    src_all = np.asarray(edge_src).astype(np.int64)
    dst_all = np.asarray(edge_dst).astype(np.int64)
    es_all = np.asarray(edge_scalars).astype(np.float32)
    sh_all = np.asarray(edge_sh).astype(np.float32)
    core_of = dst_all // NODES_PER_CORE

    in_maps = []
    for cid in range(NCORES):
        sel = np.nonzero(core_of == cid)[0]
        d = dst_all[sel]
        order = np.argsort(d, kind="stable")
        sel = sel[order]
        d = d[order]
        blk = (d - NODES_PER_CORE * cid) >> 7
        cnt = np.bincount(blk, minlength=NB)
        assert cnt.max() <= NG * 128, (cid, cnt.max())
        start = np.zeros(NB, np.int64)
        start[1:] = np.cumsum(cnt)[:-1]
        rank = np.arange(len(sel)) - start[blk]
        slot = blk * (NG * 128) + rank

        srcv = np.full(EPAD, -1, np.int64)
        srcv[slot] = src_all[sel]
        shv = np.zeros((EPAD, 4), np.float32)
        shv[slot] = sh_all[sel]
        esv = np.zeros((EPAD, 8), np.float32)
        esv[slot] = es_all[sel]
        dlv = np.zeros(EPAD, np.float32)
        dlv[slot] = (d - NODES_PER_CORE * cid - 128 * blk).astype(np.float32)

        # --- gather indices: idx = node - (GBASE - 1 - 1)... row = node+1,
        # idx = row - GBASE = node + 1 - GBASE; dummy -> DUMMY_IDX (>= 0)
        idxv = np.where(srcv >= 0, srcv + 1 - GBASE, DUMMY_IDX).astype(np.int64)
        # force the trim-order-last index of each 512-idx gather call to be
        # >= 0 by swapping that edge with a non-negative-idx edge of the SAME
        # node block (any within-block permutation is valid).
        BLKE = NG * 128
        for sgi in range(NSUPER):
            jl = (sgi + 1) * SUPER_G * 128 - 1
            if idxv[jl] >= 0:
                continue
            b0 = jl // BLKE
            cand = np.nonzero(idxv[b0 * BLKE : (b0 + 1) * BLKE] >= 0)[0]
            # exclude other supers' final slots
            cand = [b0 * BLKE + q for q in cand
                    if (b0 * BLKE + q + 1) % (SUPER_G * 128) != 0]
            assert cand, "no swap candidate in block"
            q = cand[0]
            for arr in (idxv, srcv, dlv):
                arr[jl], arr[q] = arr[q], arr[jl]
            for arr in (shv, esv):
                tmpq = arr[q].copy()
                arr[q] = arr[jl]
                arr[jl] = tmpq
        idx_g = np.tile(
            idxv.reshape(-1, 16).T.astype(np.int16), (8, 1)
        )  # wrap is uniform: IDXW*NSUPER cols total

        # es4: window w at rows 32*(w%4)+b, cols [ (w//4)*512, +512 )
        es4 = np.zeros((P, NJ * 512), np.float32)
        esw = esv.reshape(WINDOWS, 512, 8)
        for c in range(4):
            wsel = np.arange(c, WINDOWS, 4)       # these windows use strip c
            nw = len(wsel)                        # w//4 == index within wsel
            es4[32 * c : 32 * c + 8, : nw * 512] = (
                esw[wsel].transpose(2, 0, 1).reshape(8, nw * 512)
            )

        sh_t = shv.reshape(GROUPS, P, 4).transpose(1, 0, 2).reshape(P, GROUPS * 4)
        dstl = dlv.reshape(GROUPS, P).T.copy()

        in_maps.append(
            dict(
                tbl=tbl, idx_g=np.ascontiguousarray(idx_g),
                es4=np.ascontiguousarray(es4),
                sh_t=np.ascontiguousarray(sh_t),
                dstl=np.ascontiguousarray(dstl),
                w1t=w1t, w2t=w2t, iota=iota,
            )
        )
    return in_maps


def _compute_ng(edge_dst):
    dst_all = np.asarray(edge_dst).astype(np.int64)
    gblk = (dst_all // NODES_PER_CORE) * NB + ((dst_all % NODES_PER_CORE) >> 7)
    cnt = np.bincount(gblk, minlength=NB * NCORES)
    NG = int(math.ceil(cnt.max() / 128.0))
    if NG % 2:
        NG += 1
    return max(NG, 2)


def kernel(node_features, edge_src, edge_dst, edge_sh, edge_scalars, fc_w1, fc_w2):
    node_features = np.asarray(node_features, dtype=np.float32)
    edge_sh = np.asarray(edge_sh, dtype=np.float32)
    edge_scalars = np.asarray(edge_scalars, dtype=np.float32)
    fc_w1 = np.asarray(fc_w1, dtype=np.float32)
    fc_w2 = np.asarray(fc_w2, dtype=np.float32)

    NG = _compute_ng(edge_dst)
    if NG not in _PROG_CACHE:
        _PROG_CACHE[NG] = _build_program(NG)
    nc = _PROG_CACHE[NG]

    in_maps = _prep(
        node_features, edge_src, edge_dst, edge_sh, edge_scalars, fc_w1, fc_w2, NG
    )
    res = run_bass_kernel_spmd(nc, in_maps, core_ids=list(range(NCORES)))
    out = np.concatenate([res.results[c]["nodeout"] for c in range(NCORES)], axis=0)
    return out[:N_NODES].astype(np.float32)



# revision 2
# speedup vs baseline: 3.3132x; 3.3132x over previous
"""Trainium2 Bass kernel for nn_Convolution (e3nn-style GNN message passing).

Strategy (8 NeuronCores, SPMD, no collectives):
- Sort edges by destination; core c owns destination nodes [6400c, 6400(c+1)).
- Per core: edges are binned into 50 node-blocks (128 nodes each) and padded to
  NG groups of 128 edges per block. Dummy edges gather a zero table row, so
  every tensor-product output term (all bilinear in source features) is 0.
- Gather source features with dma_gather from a 256B-padded table (1024-idx
  calls covering 2 windows), split lo/hi via signed int16 offsets from GBASE.
- Radial MLP layer 1 on PE with tile_position row-packed K=8 matmuls (bf16),
  layer 2 per-group with h as the stationary operand (w lands [edge, 256]).
- TP products on DVE via broadcast APs writing bf16; the per-edge contraction
  over u is DEFERRED into the scatter matmul: one-hot(dst) x [512-wide bf16
  product tile] accumulates in PSUM over each block, reduced over u per block.
- All PE matmuls run in bf16 (4x the fp32 rate on TRN2).
"""

import math
import os
import numpy as np
import ml_dtypes

_TRACE_SIM = bool(int(os.environ.get('K_TRACE_SIM', '0')))

import concourse.bass as bass
import concourse.bacc as bacc
import concourse.mybir as mybir
from concourse.tile import TileContext
from concourse.bass_utils import run_bass_kernel_spmd

# ---------------- problem constants (hardcoded per spec) ----------------
N_NODES, N_EDGES, NUM_BASIS, HIDDEN = 50000, 800000, 8, 256
MUL = 8
INV_SQRT3 = float(1.0 / np.sqrt(3.0))
A_SCALAR = float(np.sqrt(1.0 / 128.0))
A_VECTOR = float(np.sqrt(3.0 / 128.0))
SQRT2 = float(np.sqrt(2.0))
DEG_SCALE = float(1.0 / np.sqrt(N_EDGES / N_NODES))

NCORES = 8
P = 128
NODES_PER_CORE = 6400          # 50 blocks of 128; 8*6400 = 51200 >= 50000
NB = 50                        # node blocks per core
# table: rows 1..50000 = nodes 0..49999; row 50001 = zeros (dummy target).
# gather base = row 32768, int16 idx = node - 32767 in [-32767, 17232];
# dummy idx = +17233 (always non-negative so it never hits the trailing-
# negative trim). Each gather's last (trim-order) index is forced >= 0 by an
# in-block edge swap on the host.
TBL_ROWS = 50004
GBASE = 32768
DUMMY_IDX = 50001 - GBASE

BF16 = ml_dtypes.bfloat16
_PROG_CACHE = {}


def _gather_calls(WINDOWS):
    """List of (window_start, n_windows) per gather call: pairs, last may be 1."""
    calls = []
    w = 0
    while w < WINDOWS:
        nw = 2 if w + 1 < WINDOWS else 1
        calls.append((w, nw))
        w += nw
    return calls


# ---------------- device program ----------------
def _build_program(NG):
    GROUPS = NB * NG
    WINDOWS = GROUPS // 4            # 4 groups (512 edges) per window
    IDX_COLS = GROUPS * 128 // 16    # wrapped idx cols total
    NJ = (WINDOWS + 3) // 4          # es_w4 column blocks
    ES_CHUNK_J = 3                   # es col-blocks per streamed chunk

    nc = bacc.Bacc(num_devices=NCORES, num_swdge_queues=4)
    f32, i16, bf = mybir.dt.float32, mybir.dt.int16, mybir.dt.bfloat16

    tbl = nc.dram_tensor("tbl", [TBL_ROWS, 64], f32, kind="ExternalInput")
    idx_g = nc.dram_tensor("idx_g", [P, IDX_COLS], i16, kind="ExternalInput")
    es4 = nc.dram_tensor("es4", [32, NJ * 512], bf, kind="ExternalInput")
    sh_t = nc.dram_tensor("sh_t", [P, GROUPS * 4], f32, kind="ExternalInput")
    dstl = nc.dram_tensor("dstl", [P, GROUPS], bf, kind="ExternalInput")
    w1t = nc.dram_tensor("w1t", [P, 256], bf, kind="ExternalInput")
    w2t = nc.dram_tensor("w2t", [P, 512], bf, kind="ExternalInput")
    iota = nc.dram_tensor("iota", [P, P], bf, kind="ExternalInput")
    nodeout = nc.dram_tensor("nodeout", [NODES_PER_CORE, 32], f32, kind="ExternalOutput")

    AX = mybir.AxisListType.X
    ADD = mybir.AluOpType.add
    MUL_ = mybir.AluOpType.mult
    EQ = mybir.AluOpType.is_equal
    RELU = mybir.ActivationFunctionType.Relu

    calls = _gather_calls(WINDOWS)
    call_of_w = {}
    for ci, (w0, nw) in enumerate(calls):
        for k in range(nw):
            call_of_w[w0 + k] = (ci, k)

    with TileContext(nc, trace_sim=_TRACE_SIM) as tc:
        with tc.tile_pool(name="const", bufs=1) as cpool, \
             tc.tile_pool(name="stream", bufs=2) as spool, \
             tc.tile_pool(name="work", bufs=2) as wpool, \
             tc.tile_pool(name="oh", bufs=3) as ohpool, \
             tc.tile_pool(name="psum", bufs=2, space="PSUM") as pp, \
             tc.tile_pool(name="psum1", bufs=1, space="PSUM") as pp1:

            # constants resident in SBUF
            ig_sb = cpool.tile([P, IDX_COLS], i16, name="ig")
            nc.sync.dma_start(ig_sb[:], idx_g[:])
            sh_sb = cpool.tile([P, GROUPS, 4], f32, name="sh")
            nc.sync.dma_start(sh_sb[:], sh_t[:].rearrange("p (g k) -> p g k", k=4))
            dl_sb = cpool.tile([P, GROUPS], bf, name="dl")
            nc.sync.dma_start(dl_sb[:], dstl[:])
            w1_sb = cpool.tile([P, 256], bf, name="w1")
            nc.sync.dma_start(w1_sb[:], w1t[:])
            w2_sb = cpool.tile([P, 2, 256], bf, name="w2")
            nc.sync.dma_start(w2_sb[:], w2t[:].rearrange("p (h n) -> p h n", h=2))
            io_sb = cpool.tile([P, P], bf, name="iota")
            nc.sync.dma_start(io_sb[:], iota[:])

            acc_ps = None
            x_c = None
            for w in range(WINDOWS):
                c = w % 4
                j = w // 4
                g0 = 4 * w

                # --- stream es chunk (every ES_CHUNK_J col-blocks);
                # only the 4 useful 8-row strips are loaded
                if w % (4 * ES_CHUNK_J) == 0:
                    jw = min(ES_CHUNK_J, NJ - j)
                    es_sb = spool.tile([P, ES_CHUNK_J * 512], bf, tag="es")
                    for cc in range(4):
                        nc.sync.dma_start(
                            es_sb[32 * cc : 32 * cc + 8, : jw * 512],
                            es4[8 * cc : 8 * cc + 8, j * 512 : (j + jw) * 512],
                        )
                jj = j % ES_CHUNK_J

                # --- gather: one 1024-idx call per window pair
                ci, k = call_of_w[w]
                if k == 0:
                    w0, nw = calls[ci]
                    x_c = spool.tile([P, 8, 64], f32, tag="xc")
                    nc.gpsimd.dma_gather(
                        out_ap=x_c[:, : 4 * nw, :], in_ap=tbl[GBASE:, :],
                        idxs_ap=ig_sb[:, w0 * 32 : (w0 + nw) * 32],
                        num_idxs=nw * 512, num_idxs_reg=nw * 512, elem_size=64,
                        queue_num=ci % 4,
                    )

                # --- MLP1: h[comp, edge] for 512 edges, two 128-comp halves
                h_ps = pp1.tile([P, 2, 512], f32, space="PSUM", tag="hps")
                for half in range(2):
                    nc.tensor.matmul(
                        h_ps[:, half, :],
                        lhsT=w1_sb[32 * c : 32 * c + 8, half * 128 : half * 128 + 128],
                        rhs=es_sb[32 * c : 32 * c + 8, jj * 512 : jj * 512 + 512],
                        start=True, stop=True,
                        tile_position=(32 * c, 0),
                    )
                h_sb = wpool.tile([P, 2, 512], bf, tag="hsb")
                for half in range(2):
                    nc.scalar.activation(
                        out=h_sb[:, half, :], in_=h_ps[:, half, :], func=RELU
                    )

                # --- MLP2 per group: w[edge, 256] in PSUM
                w_ps = pp.tile([P, 4, 256], f32, space="PSUM", tag="wps")
                for gg in range(4):
                    for half in range(2):
                        nc.tensor.matmul(
                            w_ps[:, gg, :],
                            lhsT=h_sb[:, half, gg * 128 : gg * 128 + 128],
                            rhs=w2_sb[:, half, :],
                            start=(half == 0), stop=(half == 1),
                        )

                # --- TP products (batched over the 4 groups)
                xs = x_c[:, 4 * k : 4 * k + 4, :]     # [P, 4, 64]
                shw = sh_sb[:, g0 : g0 + 4, :]        # [P, 4, 4]
                scat = wpool.tile([P, 4, 512], bf, tag="scat")
                ab16 = wpool.tile([P, 4, 16], f32, tag="ab16")
                # a[u] = s1[u] * s2
                nc.vector.tensor_tensor(
                    out=ab16[:, :, 0:8], in0=xs[:, :, 0:8],
                    in1=shw[:, :, 0:1].to_broadcast([P, 4, 8]), op=MUL_,
                )
                # b[u] = sum_i v1[u,i] * v2[i]
                pb = wpool.tile([P, 4, 8, 3], f32, tag="pb")
                nc.vector.tensor_tensor(
                    out=pb[:],
                    in0=xs[:, :, 8:32].rearrange("p g (u i) -> p g u i", u=8),
                    in1=shw[:, :, 1:4].unsqueeze(2).to_broadcast([P, 4, 8, 3]),
                    op=MUL_,
                )
                nc.vector.tensor_reduce(
                    out=ab16[:, :, 8:16], in_=pb[:], axis=AX, op=ADD
                )
                # ps = ab16[u'] * w01[u', w_]  -> scat[:, :, 0:128]
                nc.vector.tensor_tensor(
                    out=scat[:, :, 0:128].rearrange("p g (u w) -> p g u w", u=16),
                    in0=ab16[:].unsqueeze(3).to_broadcast([P, 4, 16, 8]),
                    in1=w_ps[:, :, 0:128].rearrange("p g (u w) -> p g u w", u=16),
                    op=MUL_,
                )
                # sv[u,i]: u 0:8 = s1[u]*v2[i], u 8:16 = v1[u,i]*s2
                sv = wpool.tile([P, 4, 16, 3], f32, tag="sv")
                nc.vector.tensor_tensor(
                    out=sv[:, :, 0:8, :],
                    in0=xs[:, :, 0:8].unsqueeze(3).to_broadcast([P, 4, 8, 3]),
                    in1=shw[:, :, 1:4].unsqueeze(2).to_broadcast([P, 4, 8, 3]),
                    op=MUL_,
                )
                nc.vector.tensor_tensor(
                    out=sv[:, :, 8:16, :].rearrange("p g u i -> p g (u i)"),
                    in0=xs[:, :, 8:32],
                    in1=shw[:, :, 0:1].to_broadcast([P, 4, 24]), op=MUL_,
                )
                # ptv = sv[u,i] * w23[u,w_] -> scat cols 128:512 ((u,w_,i))
                nc.vector.tensor_tensor(
                    out=scat[:, :, 128:512].rearrange(
                        "p g (u w i) -> p g u w i", u=16, w=8
                    ),
                    in0=sv[:].unsqueeze(3).to_broadcast([P, 4, 16, 8, 3]),
                    in1=w_ps[:, :, 128:256]
                        .rearrange("p g (u w) -> p g u w", u=16)
                        .unsqueeze(4).to_broadcast([P, 4, 16, 8, 3]),
                    op=MUL_,
                )

                # --- one-hot rows for the 4 groups in one DVE op
                oh4 = ohpool.tile([P, 4, P], bf, tag="oh")
                nc.vector.tensor_tensor(
                    out=oh4[:],
                    in0=io_sb[:].unsqueeze(1).to_broadcast([P, 4, P]),
                    in1=dl_sb[:, g0 : g0 + 4].unsqueeze(2).to_broadcast([P, 4, P]),
                    op=EQ,
                )

                # --- per group: scatter matmul into block accumulator
                for gg in range(4):
                    g = g0 + gg
                    b = g // NG
                    gib = g % NG
                    if gib == 0:
                        acc_ps = pp.tile([P, 512], f32, space="PSUM", tag="acc")
                    nc.tensor.matmul(
                        acc_ps[:],
                        lhsT=oh4[:, gg, :], rhs=scat[:, gg, :],
                        start=(gib == 0), stop=(gib == NG - 1),
                    )
                    if gib == NG - 1:
                        stage = wpool.tile([P, 32], f32, tag="stage")
                        nc.vector.tensor_reduce(
                            out=stage[:, 0:8],
                            in_=acc_ps[:, 0:128].rearrange(
                                "p (u w) -> p w u", u=16
                            ),
                            axis=AX, op=ADD,
                        )
                        nc.vector.tensor_reduce(
                            out=stage[:, 8:32],
                            in_=acc_ps[:, 128:512].rearrange(
                                "p (u wi) -> p wi u", u=16
                            ),
                            axis=AX, op=ADD,
                        )
                        nc.sync.dma_start(
                            nodeout[128 * b : 128 * b + 128, :], stage[:]
                        )
    nc.compile()
    return nc


# ---------------- host-side prep ----------------
def _prep(node_features, edge_src, edge_dst, edge_sh, edge_scalars, fc_w1, fc_w2, NG):
    GROUPS = NB * NG
    EPAD = GROUPS * 128
    WINDOWS = GROUPS // 4
    NJ = (WINDOWS + 3) // 4

    # fold all scalar coefficients into the weights
    w1s = (fc_w1 * (1.0 / math.sqrt(NUM_BASIS))).astype(np.float32)     # [8, 256]
    w2 = (fc_w2 * (SQRT2 / math.sqrt(HIDDEN))).astype(np.float64)       # [256, 256]
    w2 = w2.reshape(HIDDEN, 4, MUL, MUL)
    coef = np.array(
        [A_SCALAR, A_SCALAR * INV_SQRT3, A_VECTOR * INV_SQRT3, A_VECTOR * INV_SQRT3]
    ) * DEG_SCALE
    w2 = w2 * coef[None, :, None, None]
    # device col order: [w01 (u'16, w8) | w2 (u8, w8) | w3 (u8, w8)]
    w2dev = np.concatenate(
        [
            w2[:, 0].reshape(HIDDEN, 64),
            w2[:, 1].reshape(HIDDEN, 64),
            w2[:, 2].reshape(HIDDEN, 64),
            w2[:, 3].reshape(HIDDEN, 64),
        ],
        axis=1,
    ).astype(np.float32)                                                # [256, 256]

    w1t = np.zeros((P, 256), np.float32)
    for c in range(4):
        w1t[32 * c : 32 * c + 8] = w1s
    w2t = np.zeros((P, 512), np.float32)
    w2t[:, 0:256] = w2dev[0:128]
    w2t[:, 256:512] = w2dev[128:256]
    iota = np.broadcast_to(np.arange(P, dtype=np.float32), (P, P)).copy()

    tbl = np.zeros((TBL_ROWS, 64), np.float32)
    tbl[1 : N_NODES + 1, 0:32] = node_features

    src_all = np.asarray(edge_src).astype(np.int64)
    dst_all = np.asarray(edge_dst).astype(np.int64)
    es_all = np.asarray(edge_scalars).astype(np.float32)
    sh_all = np.asarray(edge_sh).astype(np.float32)
    core_of = dst_all // NODES_PER_CORE

    calls = _gather_calls(WINDOWS)

    in_maps = []
    for cid in range(NCORES):
        sel = np.nonzero(core_of == cid)[0]
        d = dst_all[sel]
        order = np.argsort(d, kind="stable")
        sel = sel[order]
        d = d[order]
        blk = (d - NODES_PER_CORE * cid) >> 7
        cnt = np.bincount(blk, minlength=NB)
        assert cnt.max() <= NG * 128, (cid, cnt.max())
        start = np.zeros(NB, np.int64)
        start[1:] = np.cumsum(cnt)[:-1]
        rank = np.arange(len(sel)) - start[blk]
        slot = blk * (NG * 128) + rank

        srcv = np.full(EPAD, -1, np.int64)
        srcv[slot] = src_all[sel]
        shv = np.zeros((EPAD, 4), np.float32)
        shv[slot] = sh_all[sel]
        esv = np.zeros((EPAD, 8), np.float32)
        esv[slot] = es_all[sel]
        dlv = np.zeros(EPAD, np.float32)
        dlv[slot] = (d - NODES_PER_CORE * cid - 128 * blk).astype(np.float32)

        # --- gather indices: row = node+1, idx = row - GBASE; dummy -> >= 0
        idxv = np.where(srcv >= 0, srcv + 1 - GBASE, DUMMY_IDX).astype(np.int64)
        # force the trim-order-last index of each gather call to be >= 0 by
        # swapping that edge with a non-negative-idx edge of the SAME node
        # block (any within-block permutation is valid).
        BLKE = NG * 128
        call_last = set()
        for (w0, nw) in calls:
            call_last.add((w0 + nw) * 512 - 1)
        for jl in sorted(call_last):
            if idxv[jl] >= 0:
                continue
            b0 = jl // BLKE
            cand = np.nonzero(idxv[b0 * BLKE : (b0 + 1) * BLKE] >= 0)[0]
            cand = [b0 * BLKE + q for q in cand
                    if (b0 * BLKE + q) not in call_last]
            assert cand, "no swap candidate in block"
            q = cand[0]
            for arr in (idxv, srcv, dlv):
                arr[jl], arr[q] = arr[q], arr[jl]
            for arr in (shv, esv):
                tmpq = arr[q].copy()
                arr[q] = arr[jl]
                arr[jl] = tmpq
        idx_g = np.tile(
            idxv.reshape(-1, 16).T.astype(np.int16), (8, 1)
        )  # [128, EPAD/16]

        # es4: window w at rows 8*(w%4), cols [ (w//4)*512, +512 )
        es4 = np.zeros((32, NJ * 512), np.float32)
        esw = esv.reshape(WINDOWS, 512, 8)
        for c in range(4):
            wsel = np.arange(c, WINDOWS, 4)       # these windows use strip c
            nw = len(wsel)                        # w//4 == index within wsel
            es4[8 * c : 8 * c + 8, : nw * 512] = (
                esw[wsel].transpose(2, 0, 1).reshape(8, nw * 512)
            )

        sh_t = shv.reshape(GROUPS, P, 4).transpose(1, 0, 2).reshape(P, GROUPS * 4)
        dstl = dlv.reshape(GROUPS, P).T.copy()

        in_maps.append(
            dict(
                tbl=tbl, idx_g=np.ascontiguousarray(idx_g),
                es4=np.ascontiguousarray(es4).astype(BF16),
                sh_t=np.ascontiguousarray(sh_t),
                dstl=np.ascontiguousarray(dstl).astype(BF16),
                w1t=w1t.astype(BF16), w2t=w2t.astype(BF16),
                iota=iota.astype(BF16),
            )
        )
    return in_maps


def _compute_ng(edge_dst):
    dst_all = np.asarray(edge_dst).astype(np.int64)
    gblk = (dst_all // NODES_PER_CORE) * NB + ((dst_all % NODES_PER_CORE) >> 7)
    cnt = np.bincount(gblk, minlength=NB * NCORES)
    NG = int(math.ceil(cnt.max() / 128.0))
    if NG % 2:
        NG += 1
    return max(NG, 2)


def kernel(node_features, edge_src, edge_dst, edge_sh, edge_scalars, fc_w1, fc_w2):
    node_features = np.asarray(node_features, dtype=np.float32)
    edge_sh = np.asarray(edge_sh, dtype=np.float32)
    edge_scalars = np.asarray(edge_scalars, dtype=np.float32)
    fc_w1 = np.asarray(fc_w1, dtype=np.float32)
    fc_w2 = np.asarray(fc_w2, dtype=np.float32)

    NG = _compute_ng(edge_dst)
    if NG not in _PROG_CACHE:
        _PROG_CACHE[NG] = _build_program(NG)
    nc = _PROG_CACHE[NG]

    in_maps = _prep(
        node_features, edge_src, edge_dst, edge_sh, edge_scalars, fc_w1, fc_w2, NG
    )
    res = run_bass_kernel_spmd(nc, in_maps, core_ids=list(range(NCORES)))
    out = np.concatenate([res.results[c]["nodeout"] for c in range(NCORES)], axis=0)
    return out[:N_NODES].astype(np.float32)


# revision 5
# speedup vs baseline: 4.2162x; 1.2726x over previous
"""Trainium2 Bass kernel for nn_Convolution (e3nn-style GNN message passing).

Strategy (8 NeuronCores, SPMD, no collectives):
- Sort edges by destination; core c owns destination nodes [6400c, 6400(c+1)).
- Per core: 50 destination node-blocks (128 nodes each). Blocks are assigned
  to 50 schedule SLOTS by descending edge count (per core); slot k gets
  NG_k = max over cores of ceil(count/128) groups of 128 edges. This shared
  nonuniform schedule cuts padding vs a uniform max NG. Host unpermutes the
  slot-major output back to node order.
- Dummy edges carry dst label 200 (one-hot row is all-zero) so any garbage
  they contribute is multiplied by zero in the scatter matmul; gather calls
  pass a per-call index count that skips each call's trailing dummies.
- Gather source features with dma_gather (1024-idx calls covering 2 windows)
  from a bf16 256B-row table, signed int16 offsets from GBASE.
- Radial MLP layer 1 on PE with tile_position row-packed K=8 matmuls (bf16),
  layer 2 per-group with h as the stationary operand (w lands [edge, 256]).
- TP products on DVE (bf16, packed inner APs for 2x); the per-edge u
  contraction is DEFERRED into the scatter matmul: one-hot(dst) x [512-wide
  bf16 product tile] accumulates in PSUM per block, reduced once per block.
- All PE matmuls bf16; w01 copied PSUM->SBUF bf16 on the Scalar engine.
"""

import math
import os
import numpy as np
import ml_dtypes

_TRACE_SIM = bool(int(os.environ.get('K_TRACE_SIM', '0')))

import concourse.bass as bass
import concourse.bacc as bacc
import concourse.mybir as mybir
from concourse.tile import TileContext
from concourse.bass_utils import run_bass_kernel_spmd

# ---------------- problem constants (hardcoded per spec) ----------------
N_NODES, N_EDGES, NUM_BASIS, HIDDEN = 50000, 800000, 8, 256
MUL = 8
INV_SQRT3 = float(1.0 / np.sqrt(3.0))
A_SCALAR = float(np.sqrt(1.0 / 128.0))
A_VECTOR = float(np.sqrt(3.0 / 128.0))
SQRT2 = float(np.sqrt(2.0))
DEG_SCALE = float(1.0 / np.sqrt(N_EDGES / N_NODES))

NCORES = 8
P = 128
NODES_PER_CORE = 6400          # 50 blocks of 128; 8*6400 = 51200 >= 50000
NB = 50                        # node blocks (schedule slots) per core
TBL_ROWS = 50004
GBASE = 32768
DUMMY_IDX = 50001 - GBASE      # row 50001 is zeros; idx kept >= 0
DUMMY_DL = 200.0               # one-hot label that never matches iota 0..127

BF16 = ml_dtypes.bfloat16
_PROG_CACHE = {}


def _gather_calls(WINDOWS):
    calls = []
    w = 0
    while w < WINDOWS:
        nw = 2 if w + 1 < WINDOWS else 1
        calls.append((w, nw))
        w += nw
    return calls


def _compute_schedule(edge_dst):
    """Shared nonuniform schedule: per-core blocks sorted by edge count map
    to slots; slot k gets max-over-cores group count. Returns (ngs, perms,
    counts) where perms[c][k] = block id of core c at slot k and
    counts[c][k] = that block's edge count."""
    dst_all = np.asarray(edge_dst).astype(np.int64)
    gblk = (dst_all // NODES_PER_CORE) * NB + ((dst_all % NODES_PER_CORE) >> 7)
    cnt = np.bincount(gblk, minlength=NB * NCORES).reshape(NCORES, NB)
    perms = np.argsort(-cnt, axis=1, kind="stable")          # [C, NB]
    S = np.take_along_axis(cnt, perms, axis=1)               # sorted desc
    ngs = np.maximum(np.ceil(S.max(axis=0) / P).astype(int), 1)
    while ngs.sum() % 4:
        ngs[-1] += 1
    return tuple(int(x) for x in ngs), perms, S


def _call_counts(ngs, S, calls):
    """Per gather call, the max-over-cores count of leading non-dummy slots
    (each block's real edges come first within its groups)."""
    GROUPS = sum(ngs)
    goff = np.concatenate([[0], np.cumsum(ngs)])
    # real edges in the first e slots of the padded core edge array, per core
    # build per-core cumulative real-count at each group boundary is enough:
    # within slot k, real edges occupy the first S[c,k] slots of NG_k*128.
    counts = []
    for (w0, nw) in calls:
        e_start, e_end = w0 * 512, (w0 + nw) * 512
        best = 0
        for c in range(NCORES):
            # last real edge position within [0, e_end)
            last = 0
            for k in range(NB):
                base = goff[k] * P
                if base >= e_end:
                    break
                real_end = base + min(int(S[c, k]), ngs[k] * P)
                if real_end > last:
                    last = min(real_end, e_end)
            best = max(best, last - e_start)
        best = max(best, 1)
        counts.append(min(best, (e_end - e_start)))
    return counts


# ---------------- device program ----------------
def _build_program(ngs, call_counts):
    GROUPS = sum(ngs)
    WINDOWS = GROUPS // 4
    IDX_COLS = GROUPS * 128 // 16
    NJ = (WINDOWS + 3) // 4
    ES_CHUNK_J = 3

    # group -> (slot, index-in-slot) map
    goff = np.concatenate([[0], np.cumsum(ngs)])
    slot_of_g = np.zeros(GROUPS, int)
    gib_of_g = np.zeros(GROUPS, int)
    for k in range(NB):
        slot_of_g[goff[k]:goff[k + 1]] = k
        gib_of_g[goff[k]:goff[k + 1]] = np.arange(ngs[k])

    nc = bacc.Bacc(num_devices=NCORES, num_swdge_queues=4)
    f32, i16, bf = mybir.dt.float32, mybir.dt.int16, mybir.dt.bfloat16

    tbl = nc.dram_tensor("tbl", [TBL_ROWS, 128], bf, kind="ExternalInput")
    idx_g = nc.dram_tensor("idx_g", [P, IDX_COLS], i16, kind="ExternalInput")
    es4 = nc.dram_tensor("es4", [32, NJ * 512], bf, kind="ExternalInput")
    sh_t = nc.dram_tensor("sh_t", [P, GROUPS * 4], bf, kind="ExternalInput")
    dstl = nc.dram_tensor("dstl", [P, GROUPS], bf, kind="ExternalInput")
    w1t = nc.dram_tensor("w1t", [P, 256], bf, kind="ExternalInput")
    w2t = nc.dram_tensor("w2t", [P, 512], bf, kind="ExternalInput")
    iota = nc.dram_tensor("iota", [P, P], bf, kind="ExternalInput")
    nodeout = nc.dram_tensor("nodeout", [NB * P, 32], f32, kind="ExternalOutput")

    AX = mybir.AxisListType.X
    ADD = mybir.AluOpType.add
    MUL_ = mybir.AluOpType.mult
    EQ = mybir.AluOpType.is_equal
    RELU = mybir.ActivationFunctionType.Relu

    calls = _gather_calls(WINDOWS)
    call_of_w = {}
    for ci, (w0, nw) in enumerate(calls):
        for k in range(nw):
            call_of_w[w0 + k] = (ci, k)

    with TileContext(nc, trace_sim=_TRACE_SIM) as tc:
        with tc.tile_pool(name="const", bufs=1) as cpool, \
             tc.tile_pool(name="stream", bufs=2) as spool, \
             tc.tile_pool(name="work", bufs=2) as wpool, \
             tc.tile_pool(name="oh", bufs=3) as ohpool, \
             tc.tile_pool(name="psum", bufs=2, space="PSUM") as pp, \
             tc.tile_pool(name="psum1", bufs=1, space="PSUM") as pp1:

            # constants resident in SBUF
            ig_sb = cpool.tile([P, IDX_COLS], i16, name="ig")
            nc.sync.dma_start(ig_sb[:], idx_g[:])
            sh_sb = cpool.tile([P, GROUPS, 4], bf, name="sh")
            nc.sync.dma_start(sh_sb[:], sh_t[:].rearrange("p (g k) -> p g k", k=4))
            dl_sb = cpool.tile([P, GROUPS], bf, name="dl")
            nc.sync.dma_start(dl_sb[:], dstl[:])
            w1_sb = cpool.tile([P, 256], bf, name="w1")
            nc.sync.dma_start(w1_sb[:], w1t[:])
            w2_sb = cpool.tile([P, 2, 256], bf, name="w2")
            nc.sync.dma_start(w2_sb[:], w2t[:].rearrange("p (h n) -> p h n", h=2))
            io_sb = cpool.tile([P, P], bf, name="iota")
            nc.sync.dma_start(io_sb[:], iota[:])

            acc_ps = None
            x_c = None
            for w in range(WINDOWS):
                c = w % 4
                j = w // 4
                g0 = 4 * w

                # --- stream es chunk; only the 4 useful 8-row strips
                if w % (4 * ES_CHUNK_J) == 0:
                    jw = min(ES_CHUNK_J, NJ - j)
                    es_sb = spool.tile([P, ES_CHUNK_J * 512], bf, tag="es")
                    for cc in range(4):
                        nc.sync.dma_start(
                            es_sb[32 * cc : 32 * cc + 8, : jw * 512],
                            es4[8 * cc : 8 * cc + 8, j * 512 : (j + jw) * 512],
                        )
                jj = j % ES_CHUNK_J

                # --- gather: one 1024-idx call per window pair, trailing
                # dummies trimmed via the runtime count
                ci, k = call_of_w[w]
                if k == 0:
                    w0, nw = calls[ci]
                    x_c = spool.tile([P, 8, 128], bf, tag="xc")
                    nc.gpsimd.dma_gather(
                        out_ap=x_c[:, : 4 * nw, :], in_ap=tbl[GBASE:, :],
                        idxs_ap=ig_sb[:, w0 * 32 : (w0 + nw) * 32],
                        num_idxs=nw * 512, num_idxs_reg=call_counts[ci],
                        elem_size=128,
                        queue_num=ci % 4,
                    )

                # --- MLP1: h[comp, edge] for 512 edges, two 128-comp halves
                h_ps = pp1.tile([P, 2, 512], f32, space="PSUM", tag="hps")
                for half in range(2):
                    nc.tensor.matmul(
                        h_ps[:, half, :],
                        lhsT=w1_sb[32 * c : 32 * c + 8, half * 128 : half * 128 + 128],
                        rhs=es_sb[32 * c : 32 * c + 8, jj * 512 : jj * 512 + 512],
                        start=True, stop=True,
                        tile_position=(32 * c, 0),
                    )
                h_sb = wpool.tile([P, 2, 512], bf, tag="hsb")
                for half in range(2):
                    nc.scalar.activation(
                        out=h_sb[:, half, :], in_=h_ps[:, half, :], func=RELU
                    )

                # --- MLP2 per group: w[edge, 256] in PSUM
                # col order: [w01 (w8, u'16) | w23 (u16, w8)]
                w_ps = pp.tile([P, 4, 256], f32, space="PSUM", tag="wps")
                for gg in range(4):
                    for half in range(2):
                        nc.tensor.matmul(
                            w_ps[:, gg, :],
                            lhsT=h_sb[:, half, gg * 128 : gg * 128 + 128],
                            rhs=w2_sb[:, half, :],
                            start=(half == 0), stop=(half == 1),
                        )
                # w01 -> SBUF bf16 (packed inner u' so the scat0 product is 2x)
                w01c = wpool.tile([P, 4, 8, 16], bf, tag="w01c")
                nc.scalar.copy(
                    out=w01c[:],
                    in_=w_ps[:, :, 0:128].rearrange("p g (w u) -> p g w u", w=8),
                )

                # --- TP products (batched over the 4 groups)
                xs = x_c[:, 4 * k : 4 * k + 4, :]     # [P, 4, 128] (32 used)
                shw = sh_sb[:, g0 : g0 + 4, :]        # [P, 4, 4]
                scat = wpool.tile([P, 4, 512], bf, tag="scat")
                ab16 = wpool.tile([P, 4, 16], bf, tag="ab16")
                nc.vector.tensor_tensor(
                    out=ab16[:, :, 0:8], in0=xs[:, :, 0:8],
                    in1=shw[:, :, 0:1].to_broadcast([P, 4, 8]), op=MUL_,
                )
                pb = wpool.tile([P, 4, 8, 3], bf, tag="pb")
                nc.vector.tensor_tensor(
                    out=pb[:],
                    in0=xs[:, :, 8:32].rearrange("p g (u i) -> p g u i", u=8),
                    in1=shw[:, :, 1:4].unsqueeze(2).to_broadcast([P, 4, 8, 3]),
                    op=MUL_,
                )
                with nc.allow_low_precision(reason="sum of 3 bf16 products"):
                    nc.vector.tensor_reduce(
                        out=ab16[:, :, 8:16], in_=pb[:], axis=AX, op=ADD
                    )
                nc.vector.tensor_tensor(
                    out=scat[:, :, 0:128].rearrange("p g (w u) -> p g w u", w=8),
                    in0=ab16[:].unsqueeze(2).to_broadcast([P, 4, 8, 16]),
                    in1=w01c[:],
                    op=MUL_,
                )
                sv = wpool.tile([P, 4, 16, 3], bf, tag="sv")
                nc.vector.tensor_tensor(
                    out=sv[:, :, 0:8, :],
                    in0=xs[:, :, 0:8].unsqueeze(3).to_broadcast([P, 4, 8, 3]),
                    in1=shw[:, :, 1:4].unsqueeze(2).to_broadcast([P, 4, 8, 3]),
                    op=MUL_,
                )
                nc.vector.tensor_tensor(
                    out=sv[:, :, 8:16, :].rearrange("p g u i -> p g (u i)"),
                    in0=xs[:, :, 8:32],
                    in1=shw[:, :, 0:1].to_broadcast([P, 4, 24]), op=MUL_,
                )
                nc.vector.tensor_tensor(
                    out=scat[:, :, 128:512].rearrange(
                        "p g (u w i) -> p g u w i", u=16, w=8
                    ),
                    in0=sv[:].unsqueeze(3).to_broadcast([P, 4, 16, 8, 3]),
                    in1=w_ps[:, :, 128:256]
                        .rearrange("p g (u w) -> p g u w", u=16)
                        .unsqueeze(4).to_broadcast([P, 4, 16, 8, 3]),
                    op=MUL_,
                )

                # --- one-hot rows for the 4 groups in one DVE op
                oh4 = ohpool.tile([P, 4, P], bf, tag="oh")
                nc.vector.tensor_tensor(
                    out=oh4[:],
                    in0=io_sb[:].unsqueeze(1).to_broadcast([P, 4, P]),
                    in1=dl_sb[:, g0 : g0 + 4].unsqueeze(2).to_broadcast([P, 4, P]),
                    op=EQ,
                )

                # --- per group: scatter matmul into the slot accumulator
                for gg in range(4):
                    g = g0 + gg
                    b = int(slot_of_g[g])
                    gib = int(gib_of_g[g])
                    if gib == 0:
                        acc_ps = pp.tile([P, 512], f32, space="PSUM", tag="acc")
                    nc.tensor.matmul(
                        acc_ps[:],
                        lhsT=oh4[:, gg, :], rhs=scat[:, gg, :],
                        start=(gib == 0), stop=(gib == ngs[b] - 1),
                    )
                    if gib == ngs[b] - 1:
                        stage = wpool.tile([P, 32], f32, tag="stage")
                        nc.vector.tensor_reduce(
                            out=stage[:, 0:8],
                            in_=acc_ps[:, 0:128].rearrange(
                                "p (w u) -> p w u", w=8
                            ),
                            axis=AX, op=ADD,
                        )
                        nc.vector.tensor_reduce(
                            out=stage[:, 8:32],
                            in_=acc_ps[:, 128:512].rearrange(
                                "p (u wi) -> p wi u", u=16
                            ),
                            axis=AX, op=ADD,
                        )
                        nc.sync.dma_start(
                            nodeout[128 * b : 128 * b + 128, :], stage[:]
                        )
    nc.compile()
    return nc


# ---------------- host-side prep ----------------
def _prep(node_features, edge_src, edge_dst, edge_sh, edge_scalars,
          fc_w1, fc_w2, ngs, perms, S):
    GROUPS = sum(ngs)
    EPAD = GROUPS * 128
    WINDOWS = GROUPS // 4
    NJ = (WINDOWS + 3) // 4
    goff = np.concatenate([[0], np.cumsum(ngs)])

    # fold all scalar coefficients into the weights
    w1s = (fc_w1 * (1.0 / math.sqrt(NUM_BASIS))).astype(np.float32)
    w2 = (fc_w2 * (SQRT2 / math.sqrt(HIDDEN))).astype(np.float64)
    w2 = w2.reshape(HIDDEN, 4, MUL, MUL)
    coef = np.array(
        [A_SCALAR, A_SCALAR * INV_SQRT3, A_VECTOR * INV_SQRT3, A_VECTOR * INV_SQRT3]
    ) * DEG_SCALE
    w2 = w2 * coef[None, :, None, None]
    # device col order: [w01 (w8, u'16) | w23 (u16, w8)]
    w01 = np.concatenate([w2[:, 0], w2[:, 1]], axis=1)   # [H, u'16, w8]
    w01 = w01.transpose(0, 2, 1).reshape(HIDDEN, 128)    # (w, u')
    w23 = np.concatenate([w2[:, 2], w2[:, 3]], axis=1).reshape(HIDDEN, 128)
    w2dev = np.concatenate([w01, w23], axis=1).astype(np.float32)

    w1t = np.zeros((P, 256), np.float32)
    for c in range(4):
        w1t[32 * c : 32 * c + 8] = w1s
    w2t = np.zeros((P, 512), np.float32)
    w2t[:, 0:256] = w2dev[0:128]
    w2t[:, 256:512] = w2dev[128:256]
    iota = np.broadcast_to(np.arange(P, dtype=np.float32), (P, P)).copy()

    tbl = np.zeros((TBL_ROWS, 128), np.float32)
    tbl[1 : N_NODES + 1, 0:32] = node_features

    src_all = np.asarray(edge_src).astype(np.int64)
    dst_all = np.asarray(edge_dst).astype(np.int64)
    es_all = np.asarray(edge_scalars).astype(np.float32)
    sh_all = np.asarray(edge_sh).astype(np.float32)
    core_of = dst_all // NODES_PER_CORE

    calls = _gather_calls(WINDOWS)
    tbl_bf = tbl.astype(BF16)
    w1t_bf = w1t.astype(BF16)
    w2t_bf = w2t.astype(BF16)
    iota_bf = iota.astype(BF16)

    # slot offset (in edges) per slot
    slot_e0 = goff[:NB] * P

    in_maps = []
    for cid in range(NCORES):
        sel = np.nonzero(core_of == cid)[0]
        d = dst_all[sel]
        blk = (d - NODES_PER_CORE * cid) >> 7
        slot_of_blk = np.zeros(NB, int)
        slot_of_blk[perms[cid]] = np.arange(NB)
        sk = slot_of_blk[blk]                     # slot of each edge
        order = np.argsort(sk, kind="stable")
        sel = sel[order]
        d = d[order]
        sk = sk[order]
        cnts = np.bincount(sk, minlength=NB)
        start = np.zeros(NB, np.int64)
        start[1:] = np.cumsum(cnts)[:-1]
        rank = np.arange(len(sel)) - start[sk]
        slot = slot_e0[sk] + rank
        assert (rank < np.array([ngs[k] for k in sk]) * P).all()

        srcv = np.full(EPAD, -1, np.int64)
        srcv[slot] = src_all[sel]
        shv = np.zeros((EPAD, 4), np.float32)
        shv[slot] = sh_all[sel]
        esv = np.zeros((EPAD, 8), np.float32)
        esv[slot] = es_all[sel]
        dlv = np.full(EPAD, DUMMY_DL, np.float32)
        dlv[slot] = (d & 127).astype(np.float32)

        # --- gather indices
        idxv = np.where(srcv >= 0, srcv + 1 - GBASE, DUMMY_IDX).astype(np.int64)
        # force the last PROCESSED index of each call (per call_counts) >= 0
        # by swapping within the same slot's edge range
        for ci, (w0, nw) in enumerate(calls):
            cnt_i = _CALL_COUNTS_CACHE[(ngs, ci)]
            jl = w0 * 512 + cnt_i - 1
            if idxv[jl] >= 0:
                continue
            k = int(np.searchsorted(goff * P, jl, side="right")) - 1
            lo, hi = goff[k] * P, goff[k + 1] * P
            cand = np.nonzero(idxv[lo:hi] >= 0)[0]
            cand = [lo + q for q in cand if lo + q != jl]
            assert cand, "no swap candidate in slot"
            q = cand[0]
            for arr in (idxv, srcv, dlv):
                arr[jl], arr[q] = arr[q], arr[jl]
            for arr in (shv, esv):
                tmpq = arr[q].copy()
                arr[q] = arr[jl]
                arr[jl] = tmpq
        idx_g = np.tile(
            idxv.reshape(-1, 16).T.astype(np.int16), (8, 1)
        )

        # es4: window w at rows 8*(w%4), cols [ (w//4)*512, +512 )
        es4 = np.zeros((32, NJ * 512), np.float32)
        esw = esv.reshape(WINDOWS, 512, 8)
        for c in range(4):
            wsel = np.arange(c, WINDOWS, 4)
            nw = len(wsel)
            es4[8 * c : 8 * c + 8, : nw * 512] = (
                esw[wsel].transpose(2, 0, 1).reshape(8, nw * 512)
            )

        sh_t = shv.reshape(GROUPS, P, 4).transpose(1, 0, 2).reshape(P, GROUPS * 4)
        dstl = dlv.reshape(GROUPS, P).T.copy()

        in_maps.append(
            dict(
                tbl=tbl_bf, idx_g=np.ascontiguousarray(idx_g),
                es4=np.ascontiguousarray(es4).astype(BF16),
                sh_t=np.ascontiguousarray(sh_t).astype(BF16),
                dstl=np.ascontiguousarray(dstl).astype(BF16),
                w1t=w1t_bf, w2t=w2t_bf, iota=iota_bf,
            )
        )
    return in_maps


_CALL_COUNTS_CACHE = {}


def kernel(node_features, edge_src, edge_dst, edge_sh, edge_scalars, fc_w1, fc_w2):
    node_features = np.asarray(node_features, dtype=np.float32)
    edge_sh = np.asarray(edge_sh, dtype=np.float32)
    edge_scalars = np.asarray(edge_scalars, dtype=np.float32)
    fc_w1 = np.asarray(fc_w1, dtype=np.float32)
    fc_w2 = np.asarray(fc_w2, dtype=np.float32)

    ngs, perms, S = _compute_schedule(edge_dst)
    WINDOWS = sum(ngs) // 4
    calls = _gather_calls(WINDOWS)
    counts = _call_counts(ngs, S, calls)
    for ci, cnt_i in enumerate(counts):
        _CALL_COUNTS_CACHE[(ngs, ci)] = cnt_i
    key = (ngs, tuple(counts))
    if key not in _PROG_CACHE:
        _PROG_CACHE[key] = _build_program(ngs, counts)
    nc = _PROG_CACHE[key]

    in_maps = _prep(
        node_features, edge_src, edge_dst, edge_sh, edge_scalars,
        fc_w1, fc_w2, ngs, perms, S,
    )
    res = run_bass_kernel_spmd(nc, in_maps, core_ids=list(range(NCORES)))
    out = np.zeros((NCORES * NODES_PER_CORE, 32), np.float32)
    for c in range(NCORES):
        r = res.results[c]["nodeout"]            # [NB*128, 32] slot-major
        base = c * NODES_PER_CORE
        for k in range(NB):
            b = perms[c][k]
            out[base + 128 * b : base + 128 * b + 128] = r[128 * k : 128 * k + 128]
    return out[:N_NODES].astype(np.float32)
